# revision 55
# baseline (speedup 1.0000x reference)
"""PVT-style spatial-reduction attention on 8 Trainium2 NeuronCores.

Sharding: data-parallel over batch (B=8 -> one batch element per core).
Each core runs the full attention for its batch element; weights are
replicated. No collectives needed.

v2 schedule (ACT-exp is the bottleneck engine; everything hides behind it):
  - x^T arrives as 12 pipelined DMA-transpose pieces on both HWDGE rings.
  - conv2x2s2 contracts strided views of x^T directly as matmul lhsT
    (no patch materialization).
  - attention starts after ~1/4 of the prep (conv blocks 0-3, kT chunk 0,
    qT rows 0:128); remaining conv/lnT/v/kT/qproj/proj work is drip-fed
    into PE slack between score/av matmuls via a prep queue.
  - scores psum ping-pongs in 2x[128,1024] banks; av accumulates in
    2x[128,512]; per-head softmax normalization is taken off the critical
    path by releasing the av psum with a fast reciprocal + value copy,
    then broadcasting/multiplying lazily.
"""

import os
import sys
from collections import deque

import numpy as np

for _p in ("/opt/trn_rl_repo", "/root/.axon_site/_ro/trn_rl_repo"):
    if os.path.isdir(_p) and _p not in sys.path:
        sys.path.append(_p)

import concourse.bacc as bacc
import concourse.bass as bass
import concourse.mybir as mybir
import concourse.tile as tile
from concourse.bass_utils import run_bass_kernel_spmd
from concourse.masks import make_identity

F16 = mybir.dt.float16
F32 = mybir.dt.float32

N = 4096          # q tokens (H*W = 64*64)
C = 320           # model dim
NH = 5            # heads
HD = 64           # head dim
NP = 1024         # kv tokens ((H/2)*(W/2))
QB = 512
LN_EPS = 1e-3
SCALE = HD ** -0.5
EXP_BIAS = -3.0   # constant shift inside exp; cancels in softmax

# contraction chunks over C=320: three 128-row tiles; the last one holds
# c 192:320 and uses rows 64:128 (its top 64 rows overlap chunk 1).
CCHUNKS = [(0, 0, 128), (128, 0, 128), (192, 64, 128)]  # (c_start, row0, rows)
# output chunks over C=320
OCHUNKS = [(0, 128), (128, 128), (256, 64)]


def build_bass(dbg=False):
    nc = bacc.Bacc("TRN2", target_bir_lowering=False, debug=False, num_devices=8)

    xdt_d = nc.declare_dram_parameter("xdt", [C, N], F16, isOutput=False)
    # host-packed weight blobs (few big DMAs with large per-partition lines;
    # ~27 separate small-row loads ran at descriptor-overhead rates)
    wba_d = nc.declare_dram_parameter("wba", [128, 4800], F16, isOutput=False)
    wbb_d = nc.declare_dram_parameter("wbb", [128, 2880], F16, isOutput=False)
    wb32_d = nc.declare_dram_parameter("wb32", [128, 963], F32, isOutput=False)
    # output stays in xd token order and f16: contiguous 640B rows coalesce
    # in the DMA engines (the scattered f32 write was descriptor-bound and
    # left a ~60us tail); the host casts + un-permutes.
    out_d = nc.declare_dram_parameter("out", [N, C], F16, isOutput=True)
    dbg_d = {}
    if dbg:
        for nm, shp in [("dbg_xtd0", [128, N]),
                        ("dbg_ln0", [128, NP]), ("dbg_kt0", [128, NP]),
                        ("dbg_qt0", [128, N]), ("dbg_v", [128, 8 * NH * 128]),
                        ("dbg_se", [128, 1024]), ("dbg_at", [128, 1024])]:
            dbg_d[nm] = nc.declare_dram_parameter(nm, shp, F16, isOutput=True)

    with tile.TileContext(nc) as tc:
        with (
            tc.tile_pool(name="consts", bufs=1) as consts,
            tc.tile_pool(name="wpool", bufs=1) as wpool,
            tc.tile_pool(name="big", bufs=1) as bigp,
            tc.tile_pool(name="sexp", bufs=18) as sexp_p,
            tc.tile_pool(name="attn", bufs=2) as attn_p,
            tc.tile_pool(name="small", bufs=4) as small_p,
            tc.tile_pool(name="vcop", bufs=4) as vcop_p,
            tc.tile_pool(name="outp", bufs=4) as out_p,
            tc.tile_pool(name="ps_s", bufs=2, space="PSUM") as ps_s,
            tc.tile_pool(name="ps_a", bufs=2, space="PSUM") as ps_a,
            tc.tile_pool(name="ps_m", bufs=2, space="PSUM") as ps_m,
        ):
            # ---------------- DMA: x^T pieces + weights ----------------
            # x^T tiles (c on partitions), filled by 12 transpose pieces
            # (3 c-chunks x 4 token-quarters) so conv/qproj can start after
            # the first quarter instead of after the whole 14us transpose.
            # xTd is the shift-deinterleaved copy (host-permuted xd input,
            # rows = [it, dh, dw, h'%4, w']): the conv's stationary operand
            # becomes a plain contiguous 2D slice (PE weights allow only one
            # free dim). xTd pieces go on the sync ring, xT on the ACT ring.
            # The shift-deinterleaved xd^T is pre-transposed on the host, so
            # all input DMAs are plain row-contiguous transfers. q-proj also
            # consumes the xd token order; the output DMA scatters rows back
            # to original order (same descriptor count either way).
            # xTdp[ci][sp] holds xd tokens [sp*1024, (sp+1)*1024), chunk ci.
            xTdp = [[bigp.tile([128, 1024], F16, name=f"xTd{i}_{sp}")
                     for sp in range(4)] for i in range(3)]

            def xtd_piece(sp):
                for ci, (c0, _r0, _rows) in enumerate(CCHUNKS):
                    nc.sync.dma_start(
                        out=xTdp[ci][sp],
                        in_=xdt_d[c0:c0 + 128, sp * 1024:(sp + 1) * 1024])

            # ALL DMAs issue on the sync ring: a dma_start on the ACT engine
            # blocks the ACT FIFO on ring backpressure, starving the exps.
            wba1 = wpool.tile([128, 3840], F16, name="wba1")
            wba2 = wpool.tile([128, 960], F16, name="wba2")
            wbb = wpool.tile([128, 2880], F16, name="wbb")
            wb32 = wpool.tile([128, 963], F32, name="wb32")
            xtd_piece(0)
            nc.sync.dma_start(out=wba1, in_=wba_d[:, 0:3840])
            nc.sync.dma_start(out=wba2, in_=wba_d[:, 3840:4800])
            nc.sync.dma_start(out=wb32, in_=wb32_d[:, :])
            xtd_piece(1)
            nc.sync.dma_start(out=wbb, in_=wbb_d[:, :])
            xtd_piece(2)
            xtd_piece(3)

            srw_sb = [[wba1[:, (s * 3 + ci) * C:(s * 3 + ci + 1) * C]
                       for ci in range(3)] for s in range(4)]
            wq_sb = [wba2[:, ci * C:(ci + 1) * C] for ci in range(3)]
            wk_sb = [wbb[:, ci * C:(ci + 1) * C] for ci in range(3)]
            wv_sb = [wbb[:, (3 + ci) * C:(4 + ci) * C] for ci in range(3)]
            wp_o = [wbb[0:osz, (6 + i) * C:(7 + i) * C]
                    for i, (_o0, osz) in enumerate(OCHUNKS)]
            srb_bc = wb32[:, 0:C]
            bv_bc = wb32[:, C:2 * C]
            bp_bc = wb32[:, 2 * C:3 * C]
            bk_col = [wb32[0:osz, 3 * C + i:3 * C + i + 1]
                      for i, (_o0, osz) in enumerate(OCHUNKS)]

            ident = consts.tile([128, 128], F16, name="ident")
            make_identity(nc, ident)
            eps_t = consts.tile([128, 1], F32, name="eps_t")
            nc.vector.memset(eps_t, LN_EPS)
            ebias_t = consts.tile([128, 1], F32, name="ebias_t")
            nc.vector.memset(ebias_t, EXP_BIAS)

            # warm the ACT exp table set during the ramp so the ~2.7us
            # table load doesn't land inside the attention phase.
            warm = small_p.tile([128, 1], F16, name="warm", tag="st")
            nc.scalar.activation(warm, eps_t,
                                 mybir.ActivationFunctionType.Exp)

            # v augmented: [128, kv_chunk(8), head(5), 128] with ones col 0
            # (softmax denominators land on psum partition 0), zeros 1:64,
            # v at 64:128.
            v_aug = bigp.tile([128, 8, NH, 128], F16, name="v_aug")
            nc.vector.memset(v_aug[:, :, :, 0:64], 0.0)
            nc.vector.memset(v_aug[:, :, :, 0:1], 1.0)

            lnT = [bigp.tile([128, NP], F16, name=f"lnT{i}") for i in range(3)]
            kT = [bigp.tile([osz, NP], F16, name=f"kT{i}")
                  for i, (_o0, osz) in enumerate(OCHUNKS)]
            qT = [bigp.tile([osz, N], F16, name=f"qT{i}")
                  for i, (_o0, osz) in enumerate(OCHUNKS)]

            ln_tiles = [None] * 8

            # ---------------- prep building blocks ----------------
            def conv_group(it, s):
                """One shift (dh,dw) of conv block it: 3 accumulating mms.
                xTd columns are [it(8), shift(4), tok'(128)] so the
                stationary operand is a contiguous 2D slice. conv psum uses
                the (ramp-idle) av tag so lnT transposes don't contend."""
                if s == 0:
                    conv_group.pc = ps_a.tile([128, C], F32, name="pc", tag="a")
                pc = conv_group.pc
                t0 = it * 512 + s * 128
                sp, tc0 = t0 // 1024, t0 % 1024
                for ci, (_c0, r0, rows) in enumerate(CCHUNKS):
                    nc.tensor.matmul(pc, xTdp[ci][sp][r0:128, tc0:tc0 + 128],
                                     srw_sb[s][ci][r0:128, :],
                                     start=(s == 0 and ci == 0),
                                     stop=(s == 3 and ci == 2))
                if s == 3:
                    ln_stats(it, pc)

            I32 = mybir.dt.int32

            ln_mid = [None] * 8

            def ln_stats(it, pc):
                # single DVE op moves conv out of PSUM (+bias) so the conv
                # psum slot frees immediately; the LN chain then runs off
                # the SBUF copy without gating the next conv block.
                cs = small_p.tile([128, C], F32, name="cs", tag="cvs", bufs=3)
                nc.vector.tensor_add(cs, pc, srb_bc)
                stats = small_p.tile([128, 6], F32, name="stats", tag="st")
                nc.vector.bn_stats(stats, cs)
                mv = small_p.tile([128, 2], F32, name="mv", tag="mv", bufs=3)
                nc.vector.bn_aggr(mv, stats)
                # rstd = rsqrt(var+eps) via Schraudolph seed + 2 Newton
                # steps on the (ramp-idle) GPSIMD engine: keeps ACT pure-Exp
                # (no table swaps) and keeps the DVE free for stats/ln.
                s = small_p.tile([128, 8], F32, name="nrs", tag="nr", bufs=8)
                nc.vector.tensor_scalar_add(s[:, 0:1], mv[:, 1:2], LN_EPS)
                nc.vector.tensor_scalar(
                    s[:, 1:2].bitcast(I32), s[:, 0:1].bitcast(I32),
                    1, -1,
                    op0=mybir.AluOpType.logical_shift_right,
                    op1=mybir.AluOpType.bitwise_xor)
                nc.vector.tensor_scalar_add(
                    s[:, 2:3].bitcast(I32), s[:, 1:2].bitcast(I32),
                    0x5F3759DF + 1)
                y = s[:, 2:3]
                for c in (7,):
                    nc.vector.tensor_mul(s[:, 3:4], y, y)
                    nc.vector.tensor_mul(s[:, 5:6], s[:, 3:4], s[:, 0:1])
                    nc.vector.tensor_scalar(
                        s[:, 6:7], s[:, 5:6], -0.5, 1.5,
                        op0=mybir.AluOpType.mult, op1=mybir.AluOpType.add)
                    nc.vector.tensor_mul(s[:, c:c + 1], s[:, 6:7], y)
                    y = s[:, c:c + 1]
                ln_mid[it] = (cs, mv, y)

            def ln_finish(it):
                cs, mv, y = ln_mid[it]
                ln_h = small_p.tile([128, C], F16, name="ln_h", tag="lnf")
                nc.vector.tensor_scalar(ln_h, cs, mv[:, 0:1], y,
                                        op0=mybir.AluOpType.subtract,
                                        op1=mybir.AluOpType.mult)
                ln_tiles[it] = ln_h

            def emit_lnT(it):
                # psum->sbuf copies ride the ramp-idle ACT engine ('copy' is
                # in the exp table set, so no table swap).
                ln_h = ln_tiles[it]
                for ci, (c0, _r0, _rows) in enumerate(CCHUNKS):
                    pt = ps_m.tile([128, 128], F16, name="pt", tag="m")
                    nc.tensor.transpose(pt, ln_h[:, c0:c0 + 128], ident)
                    nc.scalar.copy(lnT[ci][:, it * 128:(it + 1) * 128], pt)

            def emit_v(it):
                pv = ps_m.tile([128, C], F32, name="pv", tag="m")
                for ci, (_c0, r0, rows) in enumerate(CCHUNKS):
                    nc.tensor.matmul(pv, lnT[ci][r0:128, it * 128:(it + 1) * 128],
                                     wv_sb[ci][r0:128, :],
                                     start=(ci == 0), stop=(ci == 2))
                nc.vector.tensor_add(
                    v_aug[:, it, :, 64:],
                    pv.rearrange("p (h d) -> p h d", h=NH),
                    bv_bc.rearrange("p (h d) -> p h d", h=NH))

            def emit_kT(i, b, tag, w=QB):
                """kT[i] columns [b*w, (b+1)*w)."""
                o0, osz = OCHUNKS[i]
                pk = ps_s.tile([osz, w], F32, name="pk", tag=tag) if tag == "s" \
                    else ps_m.tile([osz, w], F32, name="pk", tag=tag)
                for ci, (_c0, r0, rows) in enumerate(CCHUNKS):
                    nc.tensor.matmul(
                        pk, wk_sb[ci][r0:128, o0:o0 + osz],
                        lnT[ci][r0:128, b * w:(b + 1) * w],
                        start=(ci == 0), stop=(ci == 2))
                nc.vector.tensor_scalar_add(
                    kT[i][:, b * w:(b + 1) * w], pk, bk_col[i])

            def emit_qproj(i, nb, tag):
                o0, osz = OCHUNKS[i]
                pq = ps_s.tile([osz, QB], F32, name="pq", tag=tag) if tag == "s" \
                    else ps_m.tile([osz, QB], F32, name="pq", tag=tag)
                sp, tc0 = (nb * QB) // 1024, (nb * QB) % 1024
                for ci, (_c0, r0, rows) in enumerate(CCHUNKS):
                    nc.tensor.matmul(
                        pq, wq_sb[ci][r0:128, o0:o0 + osz],
                        xTdp[ci][sp][r0:128, tc0:tc0 + QB],
                        start=(ci == 0), stop=(ci == 2))
                nc.vector.tensor_copy(qT[i][:, nb * QB:(nb + 1) * QB], pq)

            # ---------------- attention building blocks ----------------
            attnT = {}

            def emit_scores(qb, h, k):
                ht, hr = h // 2, (h % 2) * 64
                ps = ps_s.tile([128, 2 * QB], F32, name="ps", tag="s")
                for qh in range(2):
                    nc.tensor.matmul(
                        ps[:, qh * QB:(qh + 1) * QB],
                        kT[ht][hr:hr + HD, k * 128:(k + 1) * 128],
                        qT[ht][hr:hr + HD,
                               qb * 1024 + qh * QB:qb * 1024 + (qh + 1) * QB],
                        start=True, stop=True)
                se = sexp_p.tile([128, 2 * QB], F16, name="se", tag="sexp")
                nc.scalar.activation(se, ps, mybir.ActivationFunctionType.Exp,
                                     bias=ebias_t, scale=SCALE)
                if dbg and qb == 0 and h == 0 and k == 0:
                    nc.sync.dma_start(out=dbg_d["dbg_se"][:, :], in_=se)
                return se

            def emit_av(pavs, h, k, se):
                for qh in range(2):
                    nc.tensor.matmul(
                        pavs[qh], v_aug[:, k, h, :],
                        se[:, qh * QB:(qh + 1) * QB],
                        start=(k == 0), stop=(k == 7))

            def emit_norm(qb, h, pavs):
                """Release pav fast (reciprocal + value copy), then lazily
                broadcast+multiply into attnT."""
                dst = attnT[qb][h // 2]
                dr = (h % 2) * 64
                for qh in range(2):
                    rec = small_p.tile([1, QB], F32, name="rec", tag="rc")
                    nc.vector.reciprocal_approx_fast(rec, pavs[qh][0:1, :])
                    vcp = vcop_p.tile([64, QB], F16, name="vcp", tag="vc")
                    nc.vector.tensor_copy(vcp, pavs[qh][64:128, :])
                    rb = small_p.tile([HD, QB], F32, name="rb", tag="rb")
                    nc.gpsimd.partition_broadcast(rb, rec)
                    nc.vector.tensor_mul(
                        dst[dr:dr + HD, qh * QB:(qh + 1) * QB], vcp, rb)

            def emit_proj_qs(qb, qs):
                po = ps_m.tile([128, C], F32, name="po", tag="m")
                for ci, (o0, osz) in enumerate(OCHUNKS):
                    nc.tensor.matmul(
                        po, attnT[qb][ci][:, qs * 128:(qs + 1) * 128],
                        wp_o[ci], start=(ci == 0), stop=(ci == 2))
                o_sb = out_p.tile([128, C], F16, name="o_sb", tag="o")
                nc.vector.tensor_add(o_sb, po, bp_bc)
                g = qb * 8 + qs
                nc.sync.dma_start(out=out_d[g * 128:(g + 1) * 128, :],
                                  in_=o_sb)

            # ---------------- prep queue ----------------
            prep = deque()

            def pump(n):
                for _ in range(n):
                    if prep:
                        prep.popleft()()

            # ---------------- ramp ----------------
            # PE warm-up spam while input DMAs land: the HAM clock-gate
            # needs ~3.4us of sustained matmul activity to release 2.4 GHz;
            # these dummy transposes run on nothing but the identity tile.
            warm_ps = ps_m.tile([128, 128], F32, name="warm_ps", tag="m")
            for _ in range(160):
                nc.tensor.matmul(warm_ps, ident, ident, start=True, stop=True)
            # All conv in a dense block so the HAM clock-gate stays warm;
            # lnT(it-1) interleaves so DVE LN latency hides.
            for it in range(8):
                for s in range(4):
                    conv_group(it, s)
                if it >= 1:
                    ln_finish(it - 1)
                    emit_lnT(it - 1)
            ln_finish(7)
            emit_lnT(7)
            emit_kT(0, 0, "s")
            emit_qproj(0, 0, "s")
            emit_qproj(0, 1, "s")

            # remaining prep, in dependency-safe pump order (all items are
            # dependency-free against the ramp once v(k)/kT/qT ordering is
            # respected by pump position): kT0b by h0-k4, v(k) before av(h0,k)
            prep.append(lambda: emit_kT(0, 1, "m"))
            for it in range(8):
                prep.append(lambda it=it: emit_v(it))
            prep.append(lambda: emit_kT(1, 0, "m"))
            prep.append(lambda: emit_kT(1, 1, "m"))
            prep.append(lambda: emit_qproj(1, 0, "m"))
            prep.append(lambda: emit_qproj(1, 1, "m"))
            prep.append(lambda: emit_kT(2, 0, "m"))
            prep.append(lambda: emit_kT(2, 1, "m"))
            prep.append(lambda: emit_qproj(2, 0, "m"))
            prep.append(lambda: emit_qproj(2, 1, "m"))
            for nb in range(2, 8):
                for i in range(3):
                    prep.append(lambda i=i, nb=nb: emit_qproj(i, nb, "m"))

            # ---------------- attention ----------------
            # Flat period stream: scores for period i, av for period i-2.
            # The lag means every av's exp (and every scores' psum slot) is
            # already satisfied when the PE reaches it, so matmuls pipeline
            # back-to-back instead of paying isolated fill+drain latency.
            periods = [(qb, h, k) for qb in range(4) for h in range(NH)
                       for k in range(8)]
            ses = {}
            pavs = {}

            def emit_av_step(idx):
                qb2, h2, k2 = periods[idx]
                if k2 == 0:
                    pavs[(qb2, h2)] = [
                        ps_a.tile([128, QB], F32, name="pav", tag="a")
                        for _ in range(2)]
                emit_av(pavs[(qb2, h2)], h2, k2, ses.pop((qb2, h2, k2)))
                if k2 == 7:
                    emit_norm(qb2, h2, pavs.pop((qb2, h2)))
                    if h2 == NH - 1 and qb2 < 3:
                        for qs in range(8):
                            prep.append(
                                lambda qb2=qb2, qs=qs: emit_proj_qs(qb2, qs))

            for idx, (qb, h, k) in enumerate(periods):
                if h == 0 and k == 0:
                    attnT[qb] = [
                        attn_p.tile([osz, 1024], F16, name=f"aT{qb}_{i}",
                                    tag=f"attn{i}")
                        for i, (_o0, osz) in enumerate(OCHUNKS)]
                ses[(qb, h, k)] = emit_scores(qb, h, k)
                if idx < 12 or idx % 2 == 0:
                    pump(1)
                if idx >= 2:
                    emit_av_step(idx - 2)
            emit_av_step(len(periods) - 2)
            emit_av_step(len(periods) - 1)
            pump(len(prep))
            for qs in range(8):
                emit_proj_qs(3, qs)
            if dbg:
                for sp in range(4):
                    nc.sync.dma_start(
                        out=dbg_d["dbg_xtd0"][:, sp * 1024:(sp + 1) * 1024],
                        in_=xTdp[0][sp])
                nc.sync.dma_start(out=dbg_d["dbg_ln0"][:, :], in_=lnT[0])
                nc.sync.dma_start(out=dbg_d["dbg_kt0"][0:128, :], in_=kT[0])
                nc.sync.dma_start(
                    out=dbg_d["dbg_v"][:, :],
                    in_=v_aug.rearrange("p a b c -> p (a b c)"))
                nc.sync.dma_start(out=dbg_d["dbg_qt0"][:, :], in_=qT[0])

    nc.compile()
    return nc


_CACHE = {}


def _get_nc():
    if "nc" not in _CACHE:
        _CACHE["nc"] = build_bass()
    return _CACHE["nc"]


def make_in_maps(x, Wq, Wkv, sr_w, sr_b, ln_g, ln_b, Wp, bp):
    B = x.shape[0]
    f16 = np.float16
    f32 = np.float32
    ln_g = np.asarray(ln_g, f32)
    ln_b = np.asarray(ln_b, f32)
    wk_f = np.asarray(Wkv[:, :C], f32)
    wv_f = np.asarray(Wkv[:, C:], f32)
    wq = np.ascontiguousarray(Wq, dtype=f16)
    # fold LN gamma/beta into the K/V projections:
    #   (ln*g + b) @ W = ln @ (g[:,None]*W) + b @ W
    wk = np.ascontiguousarray(ln_g[:, None] * wk_f, dtype=f16)
    wv = np.ascontiguousarray(ln_g[:, None] * wv_f, dtype=f16)
    bk = np.ascontiguousarray(ln_b @ wk_f, dtype=f32)
    bv = np.ascontiguousarray(ln_b @ wv_f, dtype=f32)
    srw = np.ascontiguousarray(np.asarray(sr_w, dtype=f16).reshape(4 * C, C))
    wp = np.ascontiguousarray(Wp, dtype=f16)
    srb = np.ascontiguousarray(sr_b, dtype=f32)
    bpv = np.ascontiguousarray(bp, dtype=f32)
    # Host-side layout prep: shift-deinterleaved xd^T (row order
    # [it(h'//4), dh, dw, h'%4, w'] <- x row (2h'+dh)*64 + 2w'+dw),
    # pre-transposed so device DMAs are plain contiguous transfers.
    # q-proj also consumes this order; the device output DMA scatters back.
    xf = np.asarray(x, dtype=f16)
    xdt = np.ascontiguousarray(
        xf.reshape(B, 8, 4, 2, 32, 2, C)         # [B, it, h'lo, dh, w', dw, C]
          .transpose(0, 6, 1, 3, 5, 2, 4)         # [B, C, it, dh, dw, h'lo, w']
          .reshape(B, C, N))
    # weight blobs: [128, cols] with all chunks packed column-wise so each
    # blob loads in one large-line DMA.
    CH = [(0, 0), (128, 0), (192, 64)]            # (c0, r0)
    wba = np.zeros((128, 4800), f16)
    for s in range(4):
        for ci, (c0, r0) in enumerate(CH):
            col = (s * 3 + ci) * C
            wba[r0:128, col:col + C] = srw[s * C + c0 + r0:s * C + c0 + 128, :]
    for ci, (c0, r0) in enumerate(CH):
        wba[:, (12 + ci) * C:(13 + ci) * C] = wq[c0:c0 + 128, :]
    wbb = np.zeros((128, 2880), f16)
    for ci, (c0, r0) in enumerate(CH):
        wbb[:, ci * C:(ci + 1) * C] = wk[c0:c0 + 128, :]
        wbb[:, (3 + ci) * C:(4 + ci) * C] = wv[c0:c0 + 128, :]
    OCH = [(0, 128), (128, 128), (256, 64)]
    for i, (o0, osz) in enumerate(OCH):
        wbb[0:osz, (6 + i) * C:(7 + i) * C] = wp[o0:o0 + osz, :]
    wb32 = np.zeros((128, 963), f32)
    wb32[:, 0:C] = srb[None, :]
    wb32[:, C:2 * C] = bv[None, :]
    wb32[:, 2 * C:3 * C] = bpv[None, :]
    for i, (o0, osz) in enumerate(OCH):
        wb32[0:osz, 3 * C + i] = bk[o0:o0 + osz]
    wba = np.ascontiguousarray(wba)
    wbb = np.ascontiguousarray(wbb)
    wb32 = np.ascontiguousarray(wb32)
    return [
        {"xdt": xdt[i], "wba": wba, "wbb": wbb, "wb32": wb32}
        for i in range(B)
    ]


def _xd_to_orig_rows():
    """orig token row for each xd-order row (device output ordering)."""
    idx = np.arange(N)
    it, s, l = idx >> 9, (idx >> 7) & 3, idx & 127
    dh, dw = s >> 1, s & 1
    return (8 * it + dh + 2 * (l >> 5)) * 64 + 2 * (l & 31) + dw


_ORIG_ROWS = _xd_to_orig_rows()


def kernel(x, Wq, Wkv, sr_w, sr_b, ln_g, ln_b, Wp, bp, H=64, W=64):
    x = np.asarray(x, dtype=np.float32)
    B = x.shape[0]
    assert x.shape == (B, N, C), x.shape
    nc = _get_nc()
    in_maps = make_in_maps(x, Wq, Wkv, sr_w, sr_b, ln_g, ln_b, Wp, bp)
    res = run_bass_kernel_spmd(nc, in_maps, core_ids=list(range(8)))
    out = np.empty((B, N, C), np.float32)
    for i in range(B):
        out[i, _ORIG_ROWS, :] = np.asarray(res.results[i]["out"], np.float32)
    return out
